# revision 70
# baseline (speedup 1.0000x reference)
"""Trainium2 Bass kernel for nn_Critic (gnn_message_passing).

Strategy (pure data-parallel over batch, 8 cores x 128 rows):

The reference attention is algebraically collapsed: for single-query
attention, q.(feat@Wk) == feat.(Wk@q), so instead of materializing
[B,N,V] key/value projections we compute a per-row 35-vector
qk[b] = ego'[b] @ (Wq @ Wk^T) and score s[b,n] = feat[b,n,:] . qk[b].
The pooled output is (softmax @ feat) @ Wv, pooling feat first (14+1
dims; the 15th basis element is the subject-id rank-1 correction
row: pool[:,14] = subj_id*sum(w), Wv row 14 = -(Wv[0]+Wv[7])).

BatchNorm (training mode, global batch stats) needs a cross-core
AllReduce of per-feature sum / sum-of-squares ([200,6] fp32).  The
stats are read as free-axis reductions of xT = u^T directly on DVE
(xT is needed for the heads anyway), so the collective triggers
~2.5us after the pooling phase ends.  Everything stat-independent
(the ego MLP, the output-bias constant) overlaps the collective's
~12us trigger->start machinery latency and its mesh execution.

Post-CC the BN affine is folded into the activations, not the
weights: y = s3 (.) x^T with t03 appended as a 129th moving column,
so each head matmul produces h_pre AND its bias b1' in one pass; the
head matmuls run in bf16 (error budget 2e-2, this adds ~4e-3).
elu(x) is composed as relu(x) + min(exp(x),1) - 1 with the -1 folded
into the scalar output bias via column sums of t_W2; relu runs on
DVE (tensor_scalar add+max) so the ACT engine only does the exp.

Scheduling notes (from perfetto traces of this environment):
 - DMA completion semaphores land ~5-7us after the transfer, so the
   input is split: loc/flag planes + the small wq/wk block first
   (they gate masks + the qk chain), remaining planes streamed under
   the score STTs, big weight blocks last (needed only post-pool).
 - Score STTs are emitted plane-major to pipeline with the DMAs.
 - The ACT function tables (Exp/Sqrt) take 1.3us to load; dummy
   activations prefetch Sqrt during the collective and Exp right
   after the affine so the reloads stay off the critical path.
 - The collective machinery also has a core-registration floor
   (~60us after first-core start on skewed launches); pre-CC compute
   hides under it when launch skew is high.
"""

import numpy as np
from contextlib import ExitStack

import concourse.bacc as bacc
import concourse.tile as tile
from concourse import mybir
import concourse.bass as bass
from concourse.bass_utils import run_bass_kernel_spmd
from concourse.masks import make_identity

B, N, V = 1024, 256, 200
NC = 8
BS = B // NC  # 128 rows per core
F32 = mybir.dt.float32
BF16 = mybir.dt.bfloat16
ALU = mybir.AluOpType
ACTF = mybir.ActivationFunctionType
SCALE = float(1.0 / np.sqrt(V))
NEG = -1.0e9
# host-side plane order in mp: loc(2) and flag(14) first for early masks
PORD = [2, 14, 0, 1, 3, 4, 5, 6, 7, 8, 9, 10, 11, 12, 13]

# W200 column layout ([200, 867])
C_WQ = 0      # [200, 21] Wq^T   (u/d/p: 7 cols each)
C_WK = 21     # [200, 35] Wk^T   (u:14, d:14, p:7)
C_W1 = 56     # [200, 600] t_W1  (3 head blocks of 200)
C_W2 = 656    # [200, 3]  t_W2
C_B1 = 659    # [200, 3]  t_b1^T
C_EW2 = 662   # [200, 200] e_W2
C_EW3 = 862   # [200, 1]  e_W3
C_EB1 = 863   # [200, 1]  e_b1^T
C_EB2 = 864   # [200, 1]  e_b2^T
C_GAM = 865   # [200, 1]  gamma
C_BET = 866   # [200, 1]  beta
W200_COLS = 867

# WS layout ([16, 2048])
# rows 0:14, cols 0:600    wv  (u/d/p Wv blocks of 200; pv rows 7:14 zero)
# row 0,    cols 600:1800  wv07 = concat(wv[0], wv[7])
# rows 0:4, cols 1800:2000 e_W1
# row 0,    cols 2000:2004 bsum4 = concat(t_b2[:,0], e_b3)
# (row 14 cols 0:600 is filled on device with -(wv[0]+wv[7]))
WS_COLS = 2048

_cache = {}


def build_nc():
    import os
    STAGE = int(os.environ.get("K_STAGE", "9"))
    nc = bacc.Bacc(None)

    # ---- kernel I/O ----
    mp = nc.dram_tensor("mp", [BS, 15 * N], F32, kind="ExternalInput")
    egoT = nc.dram_tensor("egoT", [10, BS], F32, kind="ExternalInput")
    w200 = nc.dram_tensor("w200", [200, W200_COLS], F32, kind="ExternalInput")
    ws = nc.dram_tensor("ws", [16, WS_COLS], F32, kind="ExternalInput")
    w1bf = nc.dram_tensor("w1bf", [200, 604], BF16, kind="ExternalInput")
    wqk = nc.dram_tensor("wqk", [200, 56], F32, kind="ExternalInput")
    out = nc.dram_tensor("out", [BS, 1], F32, kind="ExternalOutput")

    VC = [(0, 128), (128, 200)]  # v-dim chunks
    SEG = [('u', 14, 0), ('d', 14, 14), ('p', 7, 28)]

    with tile.TileContext(nc) as tc:
        with ExitStack() as ctx:
            sb = ctx.enter_context(tc.tile_pool(name="sb", bufs=1))
            ps = ctx.enter_context(tc.tile_pool(name="ps", bufs=3, space="PSUM"))
            ps3 = ctx.enter_context(tc.tile_pool(name="ps3", bufs=3, space="PSUM"))
            psg = ctx.enter_context(tc.tile_pool(name="psg", bufs=1, space="PSUM"))
            dram = ctx.enter_context(tc.tile_pool(name="dram", bufs=1, space="DRAM"))

            # ---------------- DMA in (staged) ----------------
            # mp columns are host-reordered as PORD so loc/flag land first
            # and score STTs can pipeline with the later chunks.  Separate
            # tiles per chunk give precise DMA dependencies.
            # loc/flag planes and the small qk weights first (they gate the
            # mask + score start), then the remaining planes streamed under
            # the scores
            mpA = sb.tile([BS, 2 * N], F32, name="mpA")
            nc.sync.dma_start(out=mpA, in_=mp[:, 0:2 * N])
            wqkA = sb.tile([128, 56], F32)
            nc.sync.dma_start(out=wqkA, in_=wqk[0:128, :])
            wqkB = sb.tile([72, 56], F32)
            nc.sync.dma_start(out=wqkB, in_=wqk[128:200, :])
            WQK = [wqkA, wqkB]
            ego_sb = sb.tile([6, BS], F32)
            nc.sync.dma_start(out=ego_sb, in_=egoT[0:6, :])
            mpB = sb.tile([BS, 4 * N], F32, name="mpB")
            nc.sync.dma_start(out=mpB, in_=mp[:, 2 * N:6 * N])
            mpC = sb.tile([BS, 4 * N], F32, name="mpC")
            nc.sync.dma_start(out=mpC, in_=mp[:, 6 * N:10 * N])
            mpD = sb.tile([BS, 5 * N], F32, name="mpD")
            nc.sync.dma_start(out=mpD, in_=mp[:, 10 * N:15 * N])
            # the rest is only needed for the xT stage / during / after the
            # collective
            ws_sb = sb.tile([16, WS_COLS], F32)
            nc.sync.dma_start(out=ws_sb, in_=ws[:])
            egoM_sb = sb.tile([4, BS], F32)
            nc.sync.dma_start(out=egoM_sb, in_=egoT[6:10, :])
            wA = sb.tile([128, W200_COLS], F32)
            nc.sync.dma_start(out=wA, in_=w200[0:128, :])
            wB = sb.tile([72, W200_COLS], F32)
            nc.sync.dma_start(out=wB, in_=w200[128:200, :])
            W2 = [wA, wB]
            wbfA = sb.tile([128, 604], BF16, name="wbfA")
            nc.sync.dma_start(out=wbfA, in_=w1bf[0:128, :])
            wbfB = sb.tile([72, 604], BF16, name="wbfB")
            nc.sync.dma_start(out=wbfB, in_=w1bf[128:200, :])
            WBF = [wbfA, wbfB]
            CHUNKS = [(mpA, 0, 2), (mpB, 2, 6), (mpC, 6, 10), (mpD, 10, 15)]

            def plane(f):
                k = PORD.index(f)
                for t, a, b in CHUNKS:
                    if a <= k < b:
                        return t[:, (k - a) * N:(k - a + 1) * N]
                raise ValueError(f)

            ident = sb.tile([128, 128], F32)
            make_identity(nc, ident)
            ones_col = sb.tile([128, 1], F32)
            nc.gpsimd.memset(ones_col, 1.0)
            ones_row = sb.tile([1, 128], F32)
            nc.gpsimd.memset(ones_row, 1.0)
            eps_col = sb.tile([128, 1], F32)
            nc.gpsimd.memset(eps_col, 1.0e-5)

            # wv ext row: row14 cols 0:600 = -(wv[0] + wv[7])
            ext_t = sb.tile([1, 600], F32)
            nc.vector.scalar_tensor_tensor(
                ext_t, ws_sb[0:1, 600:1200], -1.0,
                ws_sb[0:1, 1200:1800], op0=ALU.mult, op1=ALU.subtract)
            nc.sync.dma_start(out=ws_sb[14:15, 0:600], in_=ext_t)

            # ---------------- query chain (PE) ----------------
            # Wcomb'[6,35]: rows = ego cols 1..6 of (Wq @ Wk^T) * SCALE
            wc_ps = ps.tile([6, 35], F32, tag="sm", name="wc_ps")
            segcols = [(0, 0, 14), (7, 14, 28), (14, 28, 35)]
            for si, (qc, k0, k1) in enumerate(segcols):
                for i in range(2):
                    nc.tensor.matmul(
                        wc_ps[:, k0:k1],
                        WQK[i][:, qc + 1:qc + 7],
                        WQK[i][:, 21 + k0:21 + k1],
                        start=(i == 0), stop=(i == 1))
            wc_sb = sb.tile([6, 35], F32)
            nc.scalar.activation(wc_sb, wc_ps, ACTF.Copy, bias=0.0, scale=SCALE)

            qk_ps = ps.tile([BS, 35], F32, tag="sm", name="qk_ps")
            nc.tensor.matmul(qk_ps, ego_sb, wc_sb, start=True, stop=True)
            qk_sb = sb.tile([BS, 35], F32)
            nc.scalar.activation(qk_sb, qk_ps, ACTF.Copy, bias=0.0, scale=1.0)

            # ---------------- masks -> score accumulators ----------------
            loc, flag = plane(2), plane(14)
            subj_loc = loc[:, 0:1]
            geM = sb.tile([BS, N], F32)
            nc.vector.tensor_scalar(geM, loc, subj_loc, NEG, op0=ALU.is_ge, op1=ALU.mult)
            leM = sb.tile([BS, N], F32)
            nc.vector.tensor_scalar(leM, loc, subj_loc, NEG, op0=ALU.is_le, op1=ALU.mult)
            nfM = sb.tile([BS, N], F32)
            nc.vector.tensor_scalar(nfM, flag, 1.0e9, NEG, op0=ALU.mult, op1=ALU.add)
            acc = {}
            acc['u'] = sb.tile([BS, N], F32, tag="accu", name="accu")
            nc.vector.tensor_tensor(acc['u'], geM, nfM, op=ALU.min)
            acc['d'] = sb.tile([BS, N], F32, tag="accd", name="accd")
            nc.vector.tensor_tensor(acc['d'], leM, nfM, op=ALU.min)
            acc['p'] = sb.tile([BS, N], F32, tag="accp", name="accp")
            nc.vector.tensor_scalar(acc['p'], flag, NEG, None, op0=ALU.mult)

            # ---------------- scores (DVE STT, plane-major) ----------------
            # plane-major in DMA-arrival order so STTs pipeline with the
            # later mp chunk transfers
            for f in [2, 0, 1, 3, 4, 5, 6, 7, 8, 9, 10, 11, 12, 13]:
                for s, nf, j0 in SEG:
                    if f >= nf:
                        continue
                    nc.vector.scalar_tensor_tensor(
                        acc[s], plane(f), qk_sb[:, j0 + f:j0 + f + 1], acc[s],
                        op0=ALU.mult, op1=ALU.add)

            if STAGE <= 1:
                g_sb = sb.tile([BS, 1], F32, name="g_sb")
                nc.vector.tensor_copy(g_sb, acc['u'][:, 0:1])
                nc.sync.dma_start(out=out[:], in_=g_sb)
                return nc

            # per-segment softmax + pooling + xT, emitted in score-completion
            # order (p finishes first under plane-major emission).
            SEGORD = [('p', 7, 28, 2), ('u', 14, 0, 0), ('d', 14, 14, 1)]
            scr = sb.tile([BS, N], F32, name="scrv")
            w_t, rs_t, wsum1, pool, poolT, xT = {}, {}, {}, {}, {}, {}
            in_b = dram.tile([200, 6], F32)
            st6 = [sb.tile([c1 - c0, 6], F32, tag=f"st6{j}", name=f"st6{j}")
                   for j, (c0, c1) in enumerate(VC)]
            for s, nf, j0, si in SEGORD:
                # softmax exp (ACT) + recip (DVE)
                w_t[s] = sb.tile([BS, N], F32, tag=f"w{s}", name=f"w{s}")
                se = sb.tile([BS, 1], F32, tag=f"se{s}", name=f"se{s}")
                nc.scalar.activation(w_t[s], acc[s], ACTF.Exp, bias=0.0,
                                     scale=1.0, accum_out=se)
                seb = sb.tile([BS, 1], F32, tag=f"seb{s}", name=f"seb{s}")
                nc.vector.tensor_scalar_add(seb, se, 1.0e-30)
                rs_t[s] = sb.tile([BS, 1], F32, tag=f"rs{s}", name=f"rs{s}")
                nc.vector.reciprocal(rs_t[s], seb)
                wsum1[s] = sb.tile([BS, 1], F32, tag=f"ws{s}", name=f"ws{s}")
                nc.vector.tensor_tensor(wsum1[s], se, rs_t[s], op=ALU.mult)
                # pooled basis [128,16] (DVE)
                pool[s] = sb.tile([BS, 16], F32, tag=f"pool{s}", name=f"pool{s}")
                for f in range(nf):
                    nc.vector.scalar_tensor_tensor(
                        scr, plane(f), 1.0, w_t[s],
                        op0=ALU.mult, op1=ALU.mult,
                        accum_out=pool[s][:, f:f + 1])
                nc.vector.tensor_scalar_mul(pool[s][:, 0:nf], pool[s][:, 0:nf],
                                            rs_t[s])
                if nf < 14:
                    nc.vector.memset(pool[s][:, nf:14], 0.0)
                nc.vector.tensor_tensor(pool[s][:, 14:15], plane(0)[:, 0:1],
                                        wsum1[s], op=ALU.mult)
                nc.vector.memset(pool[s][:, 15:16], 1.0)
                # xT[s][i] = u_s^T chunk [v, b]  (PE + ACT)
                pT = ps.tile([16, BS], F32, tag="sm", name=f"pT{s}")
                nc.tensor.transpose(pT, pool[s], ident)
                poolT[s] = sb.tile([16, BS], F32, tag=f"pT{s}", name=f"pTs{s}")
                nc.scalar.activation(poolT[s], pT, ACTF.Copy, bias=0.0, scale=1.0)
                xT[s] = []
                for i, (c0, c1) in enumerate(VC):
                    xps = ps3.tile([c1 - c0, BS], F32, tag="big", name="xps")
                    nc.tensor.matmul(xps, ws_sb[0:15, si * V + c0:si * V + c1],
                                     poolT[s][0:15, :], start=True, stop=True)
                    xsb = sb.tile([c1 - c0, BS], F32, tag=f"xT{s}{i}",
                                  name=f"xT{s}{i}")
                    nc.scalar.activation(xsb, xps, ACTF.Copy, bias=0.0, scale=1.0)
                    xT[s].append(xsb)

            # batch-stat columns (DVE), after all segments so the DVE queue
            # only stalls once on the last xT roundtrip:
            # st6[v, si] = sum_b u_s[b, v]; col 3+si = sum_b u_s[b, v]^2
            for s, nf, j0, si in SEGORD:
                for j, (c0, c1) in enumerate(VC):
                    pc = c1 - c0
                    nc.vector.reduce_sum(st6[j][:, si:si + 1], xT[s][j],
                                         axis=mybir.AxisListType.X)
                    nc.vector.scalar_tensor_tensor(
                        scr[0:pc, 0:BS], xT[s][j], 1.0, xT[s][j],
                        op0=ALU.mult, op1=ALU.mult,
                        accum_out=st6[j][:, 3 + si:4 + si])

            if STAGE <= 3:
                g_sb = sb.tile([BS, 1], F32, name="g_sb")
                nc.vector.tensor_copy(g_sb, pool['u'][:, 0:1])
                nc.sync.dma_start(out=out[:], in_=g_sb)
                return nc

            for j, (c0, c1) in enumerate(VC):
                nc.sync.dma_start(out=in_b[c0:c1, :], in_=st6[j])
            out_b = dram.tile([200, 6], F32, addr_space="Shared")
            if os.environ.get("NO_CC"):
                nc.sync.dma_start(out=out_b[:], in_=in_b[:])
            else:
                nc.gpsimd.collective_compute(
                    "AllReduce", ALU.add, ins=[in_b[:]], outs=[out_b[:]],
                    replica_groups=[list(range(NC))])

            # ego MLP: q1 = relu(ego_t@eW1+eb1); q2 = relu(q1@eW2+eb2)
            q1T, q2T = [], []
            for j, (w0, w1c) in enumerate(VC):
                pc = w1c - w0
                qp = ps3.tile([pc, BS], F32, tag="big", name="qp")
                nc.tensor.matmul(qp, ws_sb[0:4, 1800 + w0:1800 + w1c],
                                 egoM_sb, start=True, stop=True)
                qs = sb.tile([pc, BS], F32, tag=f"q1T{j}", name=f"q1T{j}")
                nc.scalar.activation(qs, qp, ACTF.Relu,
                                     bias=W2[j][:, C_EB1:C_EB1 + 1], scale=1.0)
                q1T.append(qs)
            for j, (w0, w1c) in enumerate(VC):
                pc = w1c - w0
                qp = ps3.tile([pc, BS], F32, tag="big", name="qp2")
                for i in range(2):
                    nc.tensor.matmul(qp, W2[i][:, C_EW2 + w0:C_EW2 + w1c],
                                     q1T[i], start=(i == 0), stop=(i == 1))
                qs = sb.tile([pc, BS], F32, tag=f"q2T{j}", name=f"q2T{j}")
                nc.scalar.activation(qs, qp, ACTF.Relu,
                                     bias=W2[j][:, C_EB2:C_EB2 + 1], scale=1.0)
                q2T.append(qs)

            G = psg.tile([BS, 1], F32)
            # Q1 = q2 @ eW3
            for i in range(2):
                nc.tensor.matmul(G, q2T[i], W2[i][:, C_EW3:C_EW3 + 1],
                                 start=(i == 0), stop=False, skip_group_check=True)

            # bias constant: sum(b2)+eb3 - sum_kw W2[w,k]  (the elu +1 fold)
            wsp = ps.tile([1, 3], F32, tag="sm", name="wsp")
            for i in range(2):
                nc.tensor.matmul(wsp, ones_col[0:VC[i][1] - VC[i][0], :],
                                 W2[i][:, C_W2:C_W2 + 3],
                                 start=(i == 0), stop=(i == 1))
            wss = sb.tile([1, 3], F32)
            nc.vector.tensor_copy(wss, wsp)
            r1 = sb.tile([1, 1], F32)
            nc.vector.reduce_sum(r1, ws_sb[0:1, 2000:2004], axis=mybir.AxisListType.X)
            r2 = sb.tile([1, 1], F32)
            nc.vector.reduce_sum(r2, wss, axis=mybir.AxisListType.X)
            bs_tot = sb.tile([1, 1], F32)
            nc.vector.tensor_tensor(bs_tot, r1, r2, op=ALU.subtract)
            nc.tensor.matmul(G, ones_row, bs_tot, start=False, stop=False,
                             skip_group_check=True)

            # prefetch the Sqrt ACT table during the collective wait so the
            # post-CC affine doesn't eat the 1.3us table load (only one
            # function stays resident, so the heads' Exp reload is
            # unavoidable but overlaps the head matmuls)
            dum = sb.tile([1, 1], F32, name="dum")
            nc.scalar.activation(dum, eps_col[0:1, :], ACTF.Sqrt, bias=0.0,
                                 scale=1.0)

            if STAGE <= 4:
                g_sb = sb.tile([BS, 1], F32, name="g_sb")
                nc.vector.tensor_copy(g_sb, st6[0][:, 0:1])
                nc.sync.dma_start(out=out[:], in_=g_sb)
                nc.tensor.matmul(G, ones_row, bs_tot, start=False, stop=True,
                                 skip_group_check=True)
                return nc

            # ---------------- post-CC: readback stats, BN affine ----------
            s3_t, t03_t = [], []
            for j, (c0, c1) in enumerate(VC):
                pc = c1 - c0
                st = sb.tile([pc, 6], F32, tag=f"st{j}", name=f"st{j}")
                nc.sync.dma_start(out=st, in_=out_b[c0:c1, :])
                nc.vector.tensor_scalar_mul(st, st, 1.0 / B)
                sq = sb.tile([pc, 3], F32, tag=f"sqv{j}", name=f"sqv{j}")
                nc.vector.tensor_tensor(sq, st[:, 0:3], st[:, 0:3], op=ALU.mult)
                var = sb.tile([pc, 3], F32, tag=f"var{j}", name=f"var{j}")
                nc.vector.tensor_tensor(var, st[:, 3:6], sq, op=ALU.subtract)
                std = sb.tile([pc, 3], F32, tag=f"std{j}", name=f"std{j}")
                nc.scalar.activation(std, var, ACTF.Sqrt, bias=eps_col[0:pc, :],
                                     scale=1.0)
                rstd = sb.tile([pc, 3], F32, tag=f"rstd{j}", name=f"rstd{j}")
                nc.vector.reciprocal(rstd, std)
                colg = W2[j][:, C_GAM:C_GAM + 1]
                gam_b = bass.AP(tensor=colg.tensor, offset=colg.offset,
                                ap=[colg.ap[0], [0, 3]])
                colb = W2[j][:, C_BET:C_BET + 1]
                bet_b = bass.AP(tensor=colb.tensor, offset=colb.offset,
                                ap=[colb.ap[0], [0, 3]])
                s3 = sb.tile([pc, 3], F32, tag=f"s3{j}", name=f"s3{j}")
                nc.vector.tensor_tensor(s3, rstd, gam_b, op=ALU.mult)
                z3 = sb.tile([pc, 3], F32, tag=f"z3{j}", name=f"z3{j}")
                nc.vector.tensor_tensor(z3, st[:, 0:3], s3, op=ALU.mult)
                t03 = sb.tile([pc, 3], F32, tag=f"t03{j}", name=f"t03{j}")
                nc.vector.tensor_tensor(t03, bet_b, z3, op=ALU.subtract)
                s3_t.append(s3)
                t03_t.append(t03)



            if STAGE <= 5:
                g_sb = sb.tile([BS, 1], F32, name="g_sb")
                nc.vector.tensor_copy(g_sb, s3_t[0][:, 0:1])
                nc.sync.dma_start(out=out[:], in_=g_sb)
                nc.tensor.matmul(G, ones_row, bs_tot, start=False, stop=True,
                                 skip_group_check=True)
                return nc

            # ---------------- heads ----------------
            # y[s][i] = [s3_s (.) xT[s][i] | t03_s]   ([pc, 129], bf16)
            y = {}
            for si, (s, nf, j0) in enumerate(SEG):
                y[s] = []
                for i, (c0, c1) in enumerate(VC):
                    pc = c1 - c0
                    yt = sb.tile([pc, BS + 1], BF16, tag=f"y{s}{i}",
                                 name=f"y{s}{i}")
                    nc.vector.tensor_scalar_mul(yt[:, 0:BS], xT[s][i],
                                                s3_t[i][:, si:si + 1])
                    nc.vector.tensor_copy(yt[:, BS:BS + 1],
                                          t03_t[i][:, si:si + 1])
                    y[s].append(yt)

            # hp[:, 0:128] = h_pre^T, hp[:, 128] = t03 @ W1 (bias w/o b1)
            for ki, (s, nf, j0) in enumerate(SEG):
                for j, (w0, w1c) in enumerate(VC):
                    pc = w1c - w0
                    hp = ps3.tile([pc, BS + 1], F32, tag="big", name="hp")
                    for i in range(2):
                        nc.tensor.matmul(
                            hp, WBF[i][:, ki * V + w0:ki * V + w1c],
                            y[s][i], start=(i == 0), stop=(i == 1))
                    b1c = sb.tile([pc, 1], F32, tag=f"b1c{j}", name=f"b1c{j}")
                    nc.vector.tensor_tensor(b1c, hp[:, BS:BS + 1],
                                            W2[j][:, C_B1 + ki:C_B1 + ki + 1],
                                            op=ALU.add)
                    eh = sb.tile([pc, BS], F32, tag=f"eh{j}", name=f"eh{j}")
                    nc.scalar.activation(eh, hp[:, 0:BS], ACTF.Exp, bias=b1c,
                                         scale=1.0)
                    rh = sb.tile([pc, BS], F32, tag=f"rh{j}", name=f"rh{j}")
                    nc.vector.tensor_scalar(rh, hp[:, 0:BS], b1c, 0.0,
                                            op0=ALU.add, op1=ALU.max)
                    ht = sb.tile([pc, BS], BF16, tag=f"ht{j}", name=f"ht{j}")
                    nc.vector.scalar_tensor_tensor(ht, eh, 1.0, rh,
                                                   op0=ALU.min, op1=ALU.add)
                    nc.tensor.matmul(G, ht, WBF[j][:, 600 + ki:601 + ki],
                                     start=False,
                                     stop=(ki == 2 and j == 1),
                                     skip_group_check=True)

            g_sb = sb.tile([BS, 1], F32)
            nc.vector.tensor_copy(g_sb, G)
            nc.sync.dma_start(out=out[:], in_=g_sb)

    nc.finalize()
    return nc


def prep_inputs(inputs):
    """Host-side layout-only prep (shard, transpose, concat, pad)."""
    merged = np.ascontiguousarray(inputs["merged"], dtype=np.float32)
    a = np.ascontiguousarray(inputs["a"], dtype=np.float32)

    up_Wq, up_Wk, up_Wv = inputs["up_Wq"], inputs["up_Wk"], inputs["up_Wv"]
    dn_Wq, dn_Wk, dn_Wv = inputs["dn_Wq"], inputs["dn_Wk"], inputs["dn_Wv"]
    pv_Wq, pv_Wk, pv_Wv = inputs["pv_Wq"], inputs["pv_Wk"], inputs["pv_Wv"]
    t_W1, t_b1, t_W2, t_b2 = inputs["t_W1"], inputs["t_b1"], inputs["t_W2"], inputs["t_b2"]
    e_W1, e_b1, e_W2, e_b2 = inputs["e_W1"], inputs["e_b1"], inputs["e_W2"], inputs["e_b2"]
    e_W3, e_b3 = inputs["e_W3"], inputs["e_b3"]
    gamma, beta = inputs["gamma"], inputs["beta"]

    f32 = lambda x: np.ascontiguousarray(x, dtype=np.float32)

    w200 = np.zeros((200, W200_COLS), np.float32)
    w200[:, C_WQ:C_WQ + 21] = np.concatenate([up_Wq.T, dn_Wq.T, pv_Wq.T], axis=1)
    w200[:, C_WK:C_WK + 35] = np.concatenate([up_Wk.T, dn_Wk.T, pv_Wk.T], axis=1)
    w200[:, C_W1:C_W1 + 600] = np.concatenate([t_W1[0], t_W1[1], t_W1[2]], axis=1)
    w200[:, C_W2:C_W2 + 3] = t_W2[:, :, 0].T
    w200[:, C_B1:C_B1 + 3] = t_b1.T
    w200[:, C_EW2:C_EW2 + 200] = e_W2
    w200[:, C_EW3] = e_W3[:, 0]
    w200[:, C_EB1] = e_b1
    w200[:, C_EB2] = e_b2
    w200[:, C_GAM] = gamma
    w200[:, C_BET] = beta

    pvv = np.zeros((14, V), np.float32)
    pvv[0:7] = pv_Wv
    wv = np.concatenate([up_Wv, dn_Wv, pvv], axis=1)        # [14,600]
    ws = np.zeros((16, WS_COLS), np.float32)
    ws[0:14, 0:600] = wv
    ws[0, 600:1800] = np.concatenate([wv[0], wv[7]])
    ws[0:4, 1800:2000] = e_W1
    ws[0, 2000:2004] = np.concatenate([t_b2[:, 0], e_b3])

    import ml_dtypes
    w1bf = np.zeros((200, 604), dtype=ml_dtypes.bfloat16)
    w1bf[:, 0:600] = w200[:, C_W1:C_W1 + 600].astype(ml_dtypes.bfloat16)
    w1bf[:, 600:603] = w200[:, C_W2:C_W2 + 3].astype(ml_dtypes.bfloat16)
    shared = dict(w200=f32(w200), ws=f32(ws),
                  w1bf=np.ascontiguousarray(w1bf),
                  wqk=f32(w200[:, 0:56]))

    in_maps = []
    for c in range(NC):
        sh = merged[c * BS:(c + 1) * BS]                     # [128,256,15]
        mp = f32(sh.transpose(0, 2, 1)[:, PORD, :].reshape(BS, 15 * N))
        egoT = np.zeros((10, BS), np.float32)
        egoT[0:5] = sh[:, 0, 1:6].T
        egoT[5] = a[c * BS:(c + 1) * BS]
        egoT[6:9] = sh[:, 0, 3:6].T
        egoT[9] = a[c * BS:(c + 1) * BS]
        m = dict(shared)
        m["mp"] = mp
        m["egoT"] = f32(egoT)
        in_maps.append(m)
    return in_maps


def _build():
    nc = build_nc()
    if not nc.is_finalized():
        nc.finalize()
    return nc


def kernel(**inputs):
    if "nc" not in _cache:
        _cache["nc"] = _build()
    nc = _cache["nc"]
    in_maps = prep_inputs(inputs)
    r = run_bass_kernel_spmd(nc, in_maps, list(range(NC)), trace=False)
    out = np.concatenate([r.results[c]["out"] for c in range(NC)], axis=0)
    return out.reshape(-1, 1).astype(np.float32)


def kernel_profiled(inputs, trace_cores=None):
    """Like kernel() but traces execution; returns (out, BassKernelResults)."""
    if "nc" not in _cache:
        _cache["nc"] = _build()
    nc = _cache["nc"]
    in_maps = prep_inputs(inputs)
    r = run_bass_kernel_spmd(nc, in_maps, list(range(NC)), trace=True,
                             trace_cores=trace_cores)
    out = np.concatenate([r.results[c]["out"] for c in range(NC)], axis=0)
    return out.reshape(-1, 1).astype(np.float32), r
